# revision 8
# baseline (speedup 1.0000x reference)
"""Trainium2 Bass kernel for DenseNet + PWLNN (5-NN over 250 centers).

Contract: kernel(**inputs) takes FULL inputs (as in reference.setup_inputs())
and returns the FULL output [524288, 2] float32.

Strategy: pure data parallel over 8 NeuronCores (65536 samples each).
Per core, feature-major pipeline:
  H [57, 512] built by 5 tiny matmuls + tanh (ScalarE),
  scores+enc per 128-sample chunk via one matmul (lhsT = H chunk),
  exact top-5 selection via nc.vector.max (top-8) + midpoint threshold,
  Sign activation -> +/-1 mask, PE transpose, matmul vs per-center affine
  table G/2, small vector ops combine with enc.
"""

import sys
import numpy as np

sys.path.insert(0, "/opt/trn_rl_repo")

N_SMPS = 524288
N_CORES = 8
N_PER = N_SMPS // N_CORES  # 65536
D_IN = 32
GROWTH = 5
N_LAYERS = 5
D_H = D_IN + N_LAYERS * GROWTH  # 57
D_ENC = 2
N_FCNS = 250
KNN = 5
BLK = 512
NBLK = N_PER // BLK  # 128
CH = 128
NCH = BLK // CH  # 4
NSC = N_FCNS + D_ENC  # 252 (250 scores + 2 enc columns)

_cache = {}


def _build_program(nblk=NBLK):
    import concourse.bass as bass
    import concourse.tile as tile
    import concourse.tile_sem_assignment as _tsa
    from concourse import mybir

    # All SW-DGE DMAs share one semaphore lane so downstream instructions
    # carry a single DMA wait (walrus rejects multi-wait LDWEIGHTS).
    _tsa.NUM_SWDGE_GLOBAL_SEMS = 1

    f32 = mybir.dt.float32
    AF = mybir.ActivationFunctionType

    nc = bass.Bass()

    x_fm = nc.dram_tensor("x_fm", [D_IN, N_PER], f32, kind="ExternalInput")
    w_d = [
        nc.dram_tensor(f"w{i}t", [D_IN + i * GROWTH, GROWTH], f32, kind="ExternalInput")
        for i in range(N_LAYERS)
    ]
    caug_d = nc.dram_tensor("caug", [D_H, NSC], f32, kind="ExternalInput")
    rt_d = nc.dram_tensor("rtile", [CH, NSC], f32, kind="ExternalInput")
    g1_d = nc.dram_tensor("g1", [128, 6], f32, kind="ExternalInput")
    g2_d = nc.dram_tensor("g2", [N_FCNS - 128, 6], f32, kind="ExternalInput")
    q_d = nc.dram_tensor("qtile", [CH, 6], f32, kind="ExternalInput")
    id_d = nc.dram_tensor("ident", [128, 128], f32, kind="ExternalInput")
    out_d = nc.dram_tensor("out", [N_PER, D_ENC], f32, kind="ExternalOutput")

    with tile.TileContext(nc) as tc:
        with (
            tc.tile_pool(name="const", bufs=1) as constp,
            tc.tile_pool(name="hp", bufs=3) as hp,
            tc.tile_pool(name="sp", bufs=3) as sp,
            tc.tile_pool(name="smallp", bufs=4) as smallp,
            tc.tile_pool(name="outp", bufs=2) as outp,
            tc.tile_pool(name="pz", bufs=2, space=bass.MemorySpace.PSUM) as pzp,
            tc.tile_pool(name="psm", bufs=3, space=bass.MemorySpace.PSUM) as psmp,
            tc.tile_pool(name="pst", bufs=2, space=bass.MemorySpace.PSUM) as pstp,
        ):
            wt = []
            for i in range(N_LAYERS):
                t = constp.tile([D_IN + i * GROWTH, GROWTH], f32, tag=f"w{i}")
                nc.gpsimd.dma_start(t[:], w_d[i][:])
                wt.append(t)
            caug = constp.tile([D_H, NSC], f32, tag="caug")
            nc.gpsimd.dma_start(caug[:], caug_d[:])
            rt = constp.tile([CH, NSC], f32, tag="rt")
            nc.gpsimd.dma_start(rt[:], rt_d[:])
            g1 = constp.tile([128, 6], f32, tag="g1")
            nc.gpsimd.dma_start(g1[:], g1_d[:])
            g2 = constp.tile([N_FCNS - 128, 6], f32, tag="g2")
            nc.gpsimd.dma_start(g2[:], g2_d[:])
            qt = constp.tile([CH, 6], f32, tag="qt")
            nc.gpsimd.dma_start(qt[:], q_d[:])
            ident = constp.tile([128, 128], f32, tag="ident")
            nc.gpsimd.dma_start(ident[:], id_d[:])

            for b in range(nblk):
                H = hp.tile([D_H, BLK], f32, tag="H")
                nc.gpsimd.dma_start(H[0:D_IN, :], x_fm[:, b * BLK : (b + 1) * BLK])
                for i in range(N_LAYERS):
                    fi = D_IN + i * GROWTH
                    pz = pzp.tile([GROWTH, BLK], f32, tag="pz")
                    nc.tensor.matmul(pz[:], wt[i][:], H[0:fi, :], start=True, stop=True)
                    tst = hp.tile([GROWTH, BLK], f32, tag="tst")
                    nc.scalar.activation(tst[:], pz[:], AF.Tanh)
                    nc.gpsimd.dma_start(H[fi : fi + GROWTH, :], tst[:])

                outb = outp.tile([CH, NCH, D_ENC], f32, tag="outb")
                for c in range(NCH):
                    psm = psmp.tile([CH, NSC], f32, tag="psm")
                    nc.tensor.matmul(
                        psm[:], H[:, c * CH : (c + 1) * CH], caug[:],
                        start=True, stop=True,
                    )
                    ssb = sp.tile([CH, NSC], f32, tag="ssb")
                    nc.vector.tensor_add(ssb[:], psm[:], rt[:])

                    top8 = smallp.tile([CH, 8], f32, tag="top8")
                    nc.vector.max(top8[:], ssb[:, 0:N_FCNS])
                    tsum = smallp.tile([CH, 1], f32, tag="tsum")
                    nc.vector.tensor_add(tsum[:], top8[:, 4:5], top8[:, 5:6])
                    bias_t = smallp.tile([CH, 1], f32, tag="bias_t")
                    nc.vector.tensor_scalar_mul(bias_t[:], tsum[:], -0.5)

                    S = sp.tile([CH, N_FCNS], f32, tag="S")
                    nc.scalar.activation(
                        S[:], ssb[:, 0:N_FCNS], AF.Sign, bias=bias_t[:]
                    )

                    pst = pstp.tile([128, 384], f32, tag="pst")
                    nc.tensor.matmul(
                        pst[:, 0:CH], S[:, 0:128], ident[:],
                        is_transpose=True, start=True, stop=True,
                    )
                    nc.tensor.matmul(
                        pst[0 : N_FCNS - 128, 128 : 128 + CH], S[:, 128:N_FCNS],
                        ident[:], is_transpose=True, start=True, stop=True,
                    )
                    s1 = sp.tile([128, CH], f32, tag="s1")
                    nc.scalar.copy(s1[:], pst[:, 0:CH])
                    s2 = sp.tile([N_FCNS - 128, CH], f32, tag="s2")
                    nc.vector.tensor_copy(s2[:], pst[0 : N_FCNS - 128, 128 : 128 + CH])

                    nc.tensor.matmul(
                        pst[0:CH, 256:262], s1[:], g1[:], start=True, stop=False
                    )
                    nc.tensor.matmul(
                        pst[0:CH, 256:262], s2[:], g2[:], start=False, stop=True
                    )

                    tq = smallp.tile([CH, 6], f32, tag="tq")
                    nc.vector.tensor_add(tq[:], pst[0:CH, 256:262], qt[:])
                    v1 = smallp.tile([CH, 2], f32, tag="v1")
                    nc.vector.tensor_scalar_mul(v1[:], tq[:, 0:2], ssb[:, 250:251])
                    v2 = smallp.tile([CH, 2], f32, tag="v2")
                    nc.vector.tensor_scalar_mul(v2[:], tq[:, 2:4], ssb[:, 251:252])
                    v3 = smallp.tile([CH, 2], f32, tag="v3")
                    nc.vector.tensor_add(v3[:], v1[:], v2[:])
                    nc.vector.tensor_add(outb[:, c, :], v3[:], tq[:, 4:6])

                nc.sync.dma_start(
                    out_d[b * BLK : (b + 1) * BLK, :].rearrange(
                        "(c p) o -> p c o", p=CH
                    ),
                    outb[:],
                )

    _split_multi_waits(nc, mybir)
    return nc


def _split_multi_waits(nc, mybir):
    """walrus codegen allows only one sync-wait per instruction; hoist extra
    waits into standalone EventSemaphore instructions on the same engine."""
    k = 0
    for f in nc.m.functions:
        for blk in f.blocks:
            newl = []
            changed = False
            for ins in blk.instructions:
                si = ins.sync_info
                if si is not None and len(si.on_wait) > 1:
                    waits = list(si.on_wait)
                    for w in waits[:-1]:
                        ev = mybir.InstEventSemaphore(
                            name=f"WSPLIT-{k}", ins=[], outs=[]
                        )
                        k += 1
                        ev.engine = ins.engine
                        ev.sync_info = mybir.SyncInfo(on_wait=[w], on_update=[])
                        newl.append(ev)
                    ins.sync_info = mybir.SyncInfo(
                        on_wait=[waits[-1]], on_update=list(si.on_update)
                    )
                    changed = True
                newl.append(ins)
            if changed:
                blk.instructions = newl


def _host_constants(W_list, Wout, bout, ctrs, wts, offsets):
    """Build the folded constant tables (float64 accumulation, f32 output)."""
    ctrs64 = ctrs.astype(np.float64)
    Wout64 = Wout.astype(np.float64)
    bout64 = bout.astype(np.float64)
    wts64 = wts.astype(np.float64)
    off64 = offsets.astype(np.float64)

    caug = np.zeros((D_H, NSC), dtype=np.float32)
    caug[:, 0:N_FCNS] = (2.0 * (Wout64.T @ ctrs64.T)).astype(np.float32)
    caug[:, N_FCNS : N_FCNS + D_ENC] = Wout64.T.astype(np.float32)

    rrow = (2.0 * (ctrs64 @ bout64) - np.sum(ctrs64 * ctrs64, axis=1)).astype(
        np.float32
    )
    rtile = np.zeros((CH, NSC), dtype=np.float32)
    rtile[:, 0:N_FCNS] = rrow[None, :]
    rtile[:, N_FCNS : N_FCNS + D_ENC] = bout.astype(np.float32)[None, :]

    # G[c] = [w00, w01, w10, w11, b'0, b'1]; b'_o = off[c,o] - sum_i w[c,i,o]*ctr[c,i]
    G = np.zeros((N_FCNS, 6), dtype=np.float64)
    G[:, 0] = wts64[:, 0, 0]
    G[:, 1] = wts64[:, 0, 1]
    G[:, 2] = wts64[:, 1, 0]
    G[:, 3] = wts64[:, 1, 1]
    bprime = off64 - np.einsum("cio,ci->co", wts64, ctrs64)
    G[:, 4] = bprime[:, 0]
    G[:, 5] = bprime[:, 1]

    g1 = (0.5 * G[0:128]).astype(np.float32)
    g2 = (0.5 * G[128:N_FCNS]).astype(np.float32)
    q = (0.5 * G.sum(axis=0)).astype(np.float32)
    qtile = np.broadcast_to(q[None, :], (CH, 6)).copy()

    consts = {
        "caug": caug,
        "rtile": rtile,
        "g1": np.ascontiguousarray(g1),
        "g2": np.ascontiguousarray(g2),
        "qtile": qtile,
        "ident": np.eye(128, dtype=np.float32),
    }
    for i, W in enumerate(W_list):
        consts[f"w{i}t"] = np.ascontiguousarray(W.astype(np.float32).T)
    return consts


def _run(inputs, trace=False, nblk=NBLK):
    from concourse.bass_utils import run_bass_kernel_spmd

    key = ("nc", nblk)
    if key not in _cache:
        _cache[key] = _build_program(nblk)
    nc = _cache[key]

    x = np.asarray(inputs["x"], dtype=np.float32)
    W_list = [np.asarray(inputs[f"W{i}"], dtype=np.float32) for i in range(N_LAYERS)]
    consts = _host_constants(
        W_list,
        np.asarray(inputs["Wout"], dtype=np.float32),
        np.asarray(inputs["bout"], dtype=np.float32),
        np.asarray(inputs["ctrs"], dtype=np.float32),
        np.asarray(inputs["wts"], dtype=np.float32),
        np.asarray(inputs["offsets"], dtype=np.float32),
    )

    x_fm = np.ascontiguousarray(x.T)  # [32, N_SMPS]
    in_maps = []
    for core in range(N_CORES):
        m = dict(consts)
        m["x_fm"] = np.ascontiguousarray(
            x_fm[:, core * N_PER : (core + 1) * N_PER]
        )
        in_maps.append(m)

    res = run_bass_kernel_spmd(
        nc, in_maps, list(range(N_CORES)), trace=trace
    )
    outs = [res.results[c]["out"] for c in range(N_CORES)]
    full = np.concatenate(outs, axis=0).astype(np.float32)
    return full, res


def kernel(**inputs):
    full, _ = _run(inputs, trace=False)
    return full


if __name__ == "__main__":
    rng = np.random.default_rng(0)
    demo = {"x": rng.standard_normal((N_SMPS, D_IN), dtype=np.float32)}
    for i in range(N_LAYERS):
        fan_in = D_IN + i * GROWTH
        demo[f"W{i}"] = rng.standard_normal((GROWTH, fan_in), dtype=np.float32) * 0.1
    demo["Wout"] = rng.standard_normal((D_ENC, D_H), dtype=np.float32) * 0.1
    demo["bout"] = rng.standard_normal(D_ENC, dtype=np.float32) * 0.1
    demo["ctrs"] = rng.standard_normal((N_FCNS, D_ENC), dtype=np.float32)
    demo["wts"] = 1e-5 * rng.standard_normal((N_FCNS, D_ENC, D_ENC), dtype=np.float32)
    demo["offsets"] = 1e-5 * rng.standard_normal((N_FCNS, D_ENC), dtype=np.float32)
    out = kernel(**demo)
    print(out.shape, out.dtype)
